# revision 6
# baseline (speedup 1.0000x reference)
"""Trainium2 Bass kernel for nn_AverageDistanceLoss.

Math: for batch item n with class c, unit quats qp/qg and model points
pts = points[c] ([P,3]):
  d_diag[p] = |Rp pts_p - Rg pts_p|^2 = pts_p^T S pts_p,  S = (Rp-Rg)^T (Rp-Rg)
  d_sym[p]  = min_q |Rp pts_p - Rg pts_q|^2
            = n_p + min_q (n_q - 2 pts_p^T R pts_q),      R = Rp^T Rg
(rotations preserve norms, n_p = |pts_p|^2). The heavy work per symmetric
item is a [P,P] pairwise matmul + row-min; everything index/quaternion
sized (O(B*C), O(B*P)) is host-side prep for sharding.

Sharding: data-parallel over batch. Symmetric items are dealt round-robin
across the 8 cores and packed 4-per-128-partition-group so the K=5
matmuls use 4 concurrent PE row-groups (tile_position). Non-symmetric
items need only d_diag, batched into a single K=6*16 block-diagonal
matmul per core. Each core emits per-point hinge values; the final
scalar reduction happens on host.
"""

import math

import numpy as np

NUM_CLASSES = 22
MARGIN = 0.01
B = 128
P = 1024
NCORES = 8
DIAG_SLOTS = 16  # max non-sym items per core (128/8)
DIAG_K = 6 * DIAG_SLOTS + 1  # 6 quadratic-form rows per item + one margin row

_RUNNER_CACHE: dict = {}


def _quat_to_rotmat(q):
    # q: [..., 4] (w, x, y, z) -> [..., 3, 3], float64
    w, x, y, z = q[..., 0], q[..., 1], q[..., 2], q[..., 3]
    r = np.empty(q.shape[:-1] + (3, 3), dtype=np.float64)
    r[..., 0, 0] = 1 - 2 * (y * y + z * z)
    r[..., 0, 1] = 2 * (x * y - z * w)
    r[..., 0, 2] = 2 * (x * z + y * w)
    r[..., 1, 0] = 2 * (x * y + z * w)
    r[..., 1, 1] = 1 - 2 * (x * x + z * z)
    r[..., 1, 2] = 2 * (y * z - x * w)
    r[..., 2, 0] = 2 * (x * z - y * w)
    r[..., 2, 1] = 2 * (y * z + x * w)
    r[..., 2, 2] = 1 - 2 * (x * x + y * y)
    return r


def _register_pair_min_reduce():
    """Author a custom DVE op: out = min(in0, in1) elementwise, accum_out =
    min-fold of that over the free dim. One 512-cycle VectorE pass consumes
    both PSUM banks of a G row-tile (rd0 = PSUM, rd1 = SBUF staged by ACT)
    and emits the [128,1] row-min directly."""
    import concourse.dve_ops as dve_ops_mod
    from concourse.dve_ops import DveOp
    from concourse.dve_spec import C0, Spec, Src0, Src1, minn, lower as dve_lower
    from concourse.dve_uop import DveOpSpec

    name = "PAIR_MIN_REDUCE_ANT"
    if name in dve_ops_mod._SUB_OPCODE_FOR_NAME:
        return next(op for op in dve_ops_mod.OPS if op.name == name)
    spec = Spec(body=minn(Src0, Src1), accum=minn, accum_init=C0)
    row = dve_ops_mod._CUSTOM_DVE_ROW_BASE + len(dve_ops_mod.OPS)
    assert row < 0x20
    dve_ops_mod._SUB_OPCODE_FOR_NAME[name] = row
    shas = {}
    for ver in ("v3", "v4"):
        uops = dve_lower(spec, ver=ver)
        shas[ver] = DveOpSpec(name=name, opcode=row, uops=uops, rd1_en=True).sha(ver)
    op = DveOp(name, spec, subdim=False, uops_sha=shas)
    dve_ops_mod.OPS.append(op)
    dve_ops_mod.CUSTOM_DVE_SPECS[name] = spec
    return op


def _get_runner(n_groups: int, use_diag: bool):
    key = (n_groups, use_diag)
    if key in _RUNNER_CACHE:
        return _RUNNER_CACHE[key]

    import concourse.bass as bass
    import concourse.tile as tile
    from concourse import bacc, mybir
    from concourse.bass_utils import run_bass_kernel_spmd

    f32 = mybir.dt.float32
    bf16 = mybir.dt.bfloat16
    G = n_groups

    nc = bacc.Bacc("TRN2", target_bir_lowering=False, debug=False)

    ins = {}
    outs = {}
    if G > 0:
        ins["lhs"] = nc.dram_tensor("lhs", [G, 128, P], bf16, kind="ExternalInput").ap()
        ins["rhs"] = nc.dram_tensor("rhs", [G, 128, P], bf16, kind="ExternalInput").ap()
        outs["osym"] = nc.dram_tensor(
            "osym", [G, 128, 32], f32, kind="ExternalOutput"
        ).ap()
    if use_diag:
        ins["rhsd"] = nc.dram_tensor(
            "rhsd", [DIAG_K, P], f32, kind="ExternalInput"
        ).ap()
        ins["lhsd"] = nc.dram_tensor(
            "lhsd", [DIAG_K, DIAG_SLOTS], f32, kind="ExternalInput"
        ).ap()
        outs["odiag"] = nc.dram_tensor(
            "odiag", [DIAG_SLOTS, 1], f32, kind="ExternalOutput"
        ).ap()

    with tile.TileContext(nc) as tc:
        with (
            tc.tile_pool(name="big", bufs=2) as big,
            tc.tile_pool(name="route", bufs=3) as route,
            tc.tile_pool(name="small", bufs=3) as small,
            tc.tile_pool(name="psum", bufs=2, space=bass.MemorySpace.PSUM) as psum,
        ):
            if use_diag:
                rd = small.tile([DIAG_K, P], f32, tag="rd")
                nc.sync.dma_start(rd[:], ins["rhsd"][:])
                ld = small.tile([DIAG_K, DIAG_SLOTS], f32, tag="ld")
                nc.sync.dma_start(ld[:], ins["lhsd"][:])
                pd = psum.tile([DIAG_SLOTS, P], f32, tag="pg")
                for j in range(2):
                    nc.tensor.matmul(
                        pd[:, 512 * j : 512 * (j + 1)],
                        ld[:],
                        rd[:, 512 * j : 512 * (j + 1)],
                        start=True,
                        stop=True,
                    )
                hd = small.tile([DIAG_SLOTS, P], f32, tag="hd")
                dsum = small.tile([DIAG_SLOTS, 1], f32, tag="dsum")
                nc.scalar.activation(
                    hd[:],
                    pd[:],
                    mybir.ActivationFunctionType.Relu,
                    accum_out=dsum[:],
                )
                nc.sync.dma_start(outs["odiag"][:], dsum[:])

            MINOP = _register_pair_min_reduce()
            BIG = 3.0e38
            trash = None
            for g in range(G):
                L = big.tile([128, P], bf16, tag="L")
                nc.sync.dma_start(L[:], ins["lhs"][g][:])
                R = big.tile([128, P], bf16, tag="R")
                nc.sync.dma_start(R[:], ins["rhs"][g][:])
                mins = small.tile([128, 32], f32, tag="mins")
                if trash is None:
                    trash = small.tile([128, 512], f32, tag="trash")
                for m2 in range(4):  # pairs of m-tiles share one 4-bank PSUM tile
                    for s in range(4):
                        pg = psum.tile([128, 2, P], f32, tag="pg")
                        for t in range(2):
                            m = 2 * m2 + t
                            lT = L[32 * s : 32 * s + 5, 128 * m : 128 * (m + 1)]
                            for j in range(2):
                                nc.tensor.matmul(
                                    pg[:, t, 512 * j : 512 * (j + 1)],
                                    lT,
                                    R[32 * s : 32 * s + 5, 512 * j : 512 * (j + 1)],
                                    start=True,
                                    stop=True,
                                    tile_position=(32 * s, 0),
                                )
                        c2 = route.tile([128, 2, 512], f32, tag="cp")
                        nc.scalar.copy(c2[:], pg[:, :, 512:1024])
                        for t in range(2):
                            m = 2 * m2 + t
                            col = 8 * s + m
                            nc.vector._custom_dve(
                                MINOP,
                                out=trash[:],
                                in0=pg[:, t, 0:512],
                                in1=c2[:, t, :],
                                s0=BIG,
                                accum_out=mins[:, col : col + 1],
                            )
                hs = small.tile([128, 32], f32, tag="hs")
                nc.scalar.activation(
                    hs[:], mins[:], mybir.ActivationFunctionType.Relu
                )
                nc.sync.dma_start(outs["osym"][g][:], hs[:])

    nc.compile()

    def run(in_maps):
        res = run_bass_kernel_spmd(nc, in_maps, list(range(NCORES)))
        return res.results

    runner = (run, nc)
    _RUNNER_CACHE[key] = runner
    return runner


def _prepare(poses_pred, poses_target, poses_weight, points, symmetry):
    """Host-side shard prep. Returns (in_maps, n_groups, use_diag)."""
    poses_pred = np.asarray(poses_pred, dtype=np.float32)
    poses_target = np.asarray(poses_target, dtype=np.float32)
    poses_weight = np.asarray(poses_weight, dtype=np.float32)
    points = np.asarray(points, dtype=np.float32)
    symmetry = np.asarray(symmetry, dtype=np.float32)

    w = poses_weight.reshape(B, NUM_CLASSES, 4)[:, :, 0]
    has = w > 0
    valid = has.any(axis=1)
    cls = np.argmax(has, axis=1)
    sym = symmetry[cls] > 0

    rows = np.arange(B)
    qp = poses_pred.reshape(B, NUM_CLASSES, 4)[rows, cls].astype(np.float64)
    qg = poses_target.reshape(B, NUM_CLASSES, 4)[rows, cls].astype(np.float64)
    Rp = _quat_to_rotmat(qp)
    Rg = _quat_to_rotmat(qg)

    sym_items = [n for n in range(B) if valid[n] and sym[n]]
    diag_items = [n for n in range(B) if valid[n] and not sym[n]]

    S_max = max((len(sym_items[k::NCORES]) for k in range(NCORES)), default=0)
    G = (S_max + 3) // 4
    use_diag = len(diag_items) > 0

    # per-class precompute
    pts_f64 = points.astype(np.float64)  # [C, P, 3]
    nrm = (pts_f64**2).sum(-1)  # [C, P]

    in_maps = []
    for k in range(NCORES):
        im = {}
        if G > 0:
            import ml_dtypes
            lhs = np.zeros((G, 128, P), dtype=ml_dtypes.bfloat16)
            rhs = np.zeros((G, 128, P), dtype=ml_dtypes.bfloat16)
            for si, n in enumerate(sym_items[k::NCORES]):
                g, s = si // 4, si % 4
                c = cls[n]
                R = Rp[n].T @ Rg[n]
                z = pts_f64[c] @ R.T  # [P, 3]
                base = 32 * s
                lhs[g, base : base + 3, :] = -2.0 * pts_f64[c].T
                lhs[g, base + 3, :] = 1.0
                lhs[g, base + 4, :] = nrm[c] - MARGIN
                rhs[g, base : base + 3, :] = z.T
                rhs[g, base + 3, :] = nrm[c]
                rhs[g, base + 4, :] = 1.0
            im["lhs"] = lhs
            im["rhs"] = rhs
        if use_diag:
            rhsd = np.zeros((DIAG_K, P), dtype=np.float32)
            lhsd = np.zeros((DIAG_K, DIAG_SLOTS), dtype=np.float32)
            rhsd[6 * DIAG_SLOTS, :] = 1.0
            for d, n in enumerate(diag_items[k::NCORES]):
                c = cls[n]
                Sm = (Rp[n] - Rg[n]).T @ (Rp[n] - Rg[n])
                p3 = pts_f64[c]  # [P, 3]
                r = 6 * d
                rhsd[r + 0, :] = p3[:, 0] ** 2
                rhsd[r + 1, :] = p3[:, 1] ** 2
                rhsd[r + 2, :] = p3[:, 2] ** 2
                rhsd[r + 3, :] = p3[:, 0] * p3[:, 1]
                rhsd[r + 4, :] = p3[:, 0] * p3[:, 2]
                rhsd[r + 5, :] = p3[:, 1] * p3[:, 2]
                lhsd[r + 0, d] = Sm[0, 0]
                lhsd[r + 1, d] = Sm[1, 1]
                lhsd[r + 2, d] = Sm[2, 2]
                lhsd[r + 3, d] = 2.0 * Sm[0, 1]
                lhsd[r + 4, d] = 2.0 * Sm[0, 2]
                lhsd[r + 5, d] = 2.0 * Sm[1, 2]
                lhsd[6 * DIAG_SLOTS, d] = -MARGIN
            im["rhsd"] = rhsd
            im["lhsd"] = lhsd
        in_maps.append(im)
    return in_maps, G, use_diag


def kernel(poses_pred, poses_target, poses_weight, points, symmetry):
    in_maps, G, use_diag = _prepare(
        poses_pred, poses_target, poses_weight, points, symmetry
    )
    if G == 0 and not use_diag:
        return np.float32(0.0)

    run, _nc = _get_runner(G, use_diag)
    results = run(in_maps)

    total = 0.0
    for k in range(NCORES):
        if G > 0:
            total += results[k]["osym"].astype(np.float64).sum()
        if use_diag:
            total += results[k]["odiag"].astype(np.float64).sum()
    return np.float32(0.5 * total / (B * P))


# revision 7
# speedup vs baseline: 1.1308x; 1.1308x over previous
"""Trainium2 Bass kernel for nn_AverageDistanceLoss.

Math: for batch item n with class c, unit quats qp/qg and model points
pts = points[c] ([P,3]):
  d_diag[p] = |Rp pts_p - Rg pts_p|^2 = pts_p^T S pts_p,  S = (Rp-Rg)^T (Rp-Rg)
  d_sym[p]  = min_q |Rp pts_p - Rg pts_q|^2
            = n_p + min_q (n_q - 2 pts_p^T R pts_q),      R = Rp^T Rg
(rotations preserve norms, n_p = |pts_p|^2). The heavy work per symmetric
item is a [P,P] pairwise matmul + row-min; everything index/quaternion
sized (O(B*C), O(B*P)) is host-side prep for sharding.

Sharding: data-parallel over batch. Symmetric items are dealt round-robin
across the 8 cores and packed 4-per-128-partition-group so the K=5
matmuls use 4 concurrent PE row-groups (tile_position). Non-symmetric
items need only d_diag, batched into a single K=6*16 block-diagonal
matmul per core. Each core emits per-point hinge values; the final
scalar reduction happens on host.
"""

import math

import numpy as np

NUM_CLASSES = 22
MARGIN = 0.01
B = 128
P = 1024
NCORES = 8
DIAG_SLOTS = 16  # max non-sym items per core (128/8)
DIAG_K = 6 * DIAG_SLOTS + 1  # 6 quadratic-form rows per item + one margin row

_RUNNER_CACHE: dict = {}


def _quat_to_rotmat(q):
    # q: [..., 4] (w, x, y, z) -> [..., 3, 3], float64
    w, x, y, z = q[..., 0], q[..., 1], q[..., 2], q[..., 3]
    r = np.empty(q.shape[:-1] + (3, 3), dtype=np.float64)
    r[..., 0, 0] = 1 - 2 * (y * y + z * z)
    r[..., 0, 1] = 2 * (x * y - z * w)
    r[..., 0, 2] = 2 * (x * z + y * w)
    r[..., 1, 0] = 2 * (x * y + z * w)
    r[..., 1, 1] = 1 - 2 * (x * x + z * z)
    r[..., 1, 2] = 2 * (y * z - x * w)
    r[..., 2, 0] = 2 * (x * z - y * w)
    r[..., 2, 1] = 2 * (y * z + x * w)
    r[..., 2, 2] = 1 - 2 * (x * x + y * y)
    return r


def _register_pair_min_reduce():
    """Author a custom DVE op: out = min(in0, in1) elementwise, accum_out =
    min-fold of that over the free dim. One 512-cycle VectorE pass consumes
    both PSUM banks of a G row-tile (rd0 = PSUM, rd1 = SBUF staged by ACT)
    and emits the [128,1] row-min directly."""
    import concourse.dve_ops as dve_ops_mod
    from concourse.dve_ops import DveOp
    from concourse.dve_spec import C0, Spec, Src0, Src1, minn, lower as dve_lower
    from concourse.dve_uop import DveOpSpec

    name = "PAIR_MIN_REDUCE_ANT"
    if name in dve_ops_mod._SUB_OPCODE_FOR_NAME:
        return next(op for op in dve_ops_mod.OPS if op.name == name)
    spec = Spec(body=minn(Src0, Src1), accum=minn, accum_init=C0)
    row = dve_ops_mod._CUSTOM_DVE_ROW_BASE + len(dve_ops_mod.OPS)
    assert row < 0x20
    dve_ops_mod._SUB_OPCODE_FOR_NAME[name] = row
    shas = {}
    for ver in ("v3", "v4"):
        uops = dve_lower(spec, ver=ver)
        shas[ver] = DveOpSpec(name=name, opcode=row, uops=uops, rd1_en=True).sha(ver)
    op = DveOp(name, spec, subdim=False, uops_sha=shas)
    dve_ops_mod.OPS.append(op)
    dve_ops_mod.CUSTOM_DVE_SPECS[name] = spec
    return op


def _get_runner(n_groups: int, use_diag: bool):
    key = (n_groups, use_diag)
    if key in _RUNNER_CACHE:
        return _RUNNER_CACHE[key]

    import concourse.bass as bass
    import concourse.tile as tile
    from concourse import bacc, mybir
    from concourse.bass_utils import run_bass_kernel_spmd

    f32 = mybir.dt.float32
    bf16 = mybir.dt.bfloat16
    G = n_groups

    nc = bacc.Bacc("TRN2", target_bir_lowering=False, debug=False)

    ins = {}
    outs = {}
    if G > 0:
        ins["lhs"] = nc.dram_tensor("lhs", [G, 128, P], bf16, kind="ExternalInput").ap()
        ins["rhs"] = nc.dram_tensor("rhs", [G, 128, P], bf16, kind="ExternalInput").ap()
        outs["osym"] = nc.dram_tensor(
            "osym", [G, 128, 32], f32, kind="ExternalOutput"
        ).ap()
    if use_diag:
        ins["rhsd"] = nc.dram_tensor(
            "rhsd", [DIAG_K, P], f32, kind="ExternalInput"
        ).ap()
        ins["lhsd"] = nc.dram_tensor(
            "lhsd", [DIAG_K, DIAG_SLOTS], f32, kind="ExternalInput"
        ).ap()
        outs["odiag"] = nc.dram_tensor(
            "odiag", [DIAG_SLOTS, 1], f32, kind="ExternalOutput"
        ).ap()

    with tile.TileContext(nc) as tc:
        with (
            tc.tile_pool(name="big", bufs=2) as big,
            tc.tile_pool(name="route", bufs=3) as route,
            tc.tile_pool(name="small", bufs=3) as small,
            tc.tile_pool(name="psum", bufs=3, space=bass.MemorySpace.PSUM) as psum,
            tc.tile_pool(name="psumd", bufs=1, space=bass.MemorySpace.PSUM) as psumd,
        ):
            if use_diag:
                rd = small.tile([DIAG_K, P], f32, tag="rd")
                nc.sync.dma_start(rd[:], ins["rhsd"][:])
                ld = small.tile([DIAG_K, DIAG_SLOTS], f32, tag="ld")
                nc.sync.dma_start(ld[:], ins["lhsd"][:])
                pd = psumd.tile([DIAG_SLOTS, P], f32)
                for j in range(2):
                    nc.tensor.matmul(
                        pd[:, 512 * j : 512 * (j + 1)],
                        ld[:],
                        rd[:, 512 * j : 512 * (j + 1)],
                        start=True,
                        stop=True,
                    )
                hd = small.tile([DIAG_SLOTS, P], f32, tag="hd")
                dsum = small.tile([DIAG_SLOTS, 1], f32, tag="dsum")
                nc.scalar.activation(
                    hd[:],
                    pd[:],
                    mybir.ActivationFunctionType.Relu,
                    accum_out=dsum[:],
                )
                nc.sync.dma_start(outs["odiag"][:], dsum[:])

            MINOP = _register_pair_min_reduce()
            BIG = 3.0e38
            trash = None
            for g in range(G):
                L = big.tile([128, P], bf16, tag="L")
                nc.sync.dma_start(L[:], ins["lhs"][g][:])
                R = big.tile([128, P], bf16, tag="R")
                nc.sync.dma_start(R[:], ins["rhs"][g][:])
                mins = small.tile([128, 32], f32, tag="mins")
                if trash is None:
                    trash = small.tile([128, 512], f32, tag="trash")
                for m in range(8):
                    for s in range(4):
                        pg = psum.tile([128, P], f32, tag="pg")
                        lT = L[32 * s : 32 * s + 5, 128 * m : 128 * (m + 1)]
                        for j in range(2):
                            nc.tensor.matmul(
                                pg[:, 512 * j : 512 * (j + 1)],
                                lT,
                                R[32 * s : 32 * s + 5, 512 * j : 512 * (j + 1)],
                                start=True,
                                stop=True,
                                tile_position=(32 * s, 0),
                            )
                        col = 8 * s + m
                        c1 = route.tile([128, 512], f32, tag="cp")
                        nc.scalar.copy(c1[:], pg[:, 512:1024])
                        nc.vector._custom_dve(
                            MINOP,
                            out=trash[:],
                            in0=pg[:, 0:512],
                            in1=c1[:],
                            s0=BIG,
                            accum_out=mins[:, col : col + 1],
                        )
                hs = small.tile([128, 32], f32, tag="hs")
                nc.scalar.activation(
                    hs[:], mins[:], mybir.ActivationFunctionType.Relu
                )
                nc.sync.dma_start(outs["osym"][g][:], hs[:])

    nc.compile()

    def run(in_maps):
        res = run_bass_kernel_spmd(nc, in_maps, list(range(NCORES)))
        return res.results

    runner = (run, nc)
    _RUNNER_CACHE[key] = runner
    return runner


def _prepare(poses_pred, poses_target, poses_weight, points, symmetry):
    """Host-side shard prep. Returns (in_maps, n_groups, use_diag)."""
    poses_pred = np.asarray(poses_pred, dtype=np.float32)
    poses_target = np.asarray(poses_target, dtype=np.float32)
    poses_weight = np.asarray(poses_weight, dtype=np.float32)
    points = np.asarray(points, dtype=np.float32)
    symmetry = np.asarray(symmetry, dtype=np.float32)

    w = poses_weight.reshape(B, NUM_CLASSES, 4)[:, :, 0]
    has = w > 0
    valid = has.any(axis=1)
    cls = np.argmax(has, axis=1)
    sym = symmetry[cls] > 0

    rows = np.arange(B)
    qp = poses_pred.reshape(B, NUM_CLASSES, 4)[rows, cls].astype(np.float64)
    qg = poses_target.reshape(B, NUM_CLASSES, 4)[rows, cls].astype(np.float64)
    Rp = _quat_to_rotmat(qp)
    Rg = _quat_to_rotmat(qg)

    sym_items = [n for n in range(B) if valid[n] and sym[n]]
    diag_items = [n for n in range(B) if valid[n] and not sym[n]]

    S_max = max((len(sym_items[k::NCORES]) for k in range(NCORES)), default=0)
    G = (S_max + 3) // 4
    use_diag = len(diag_items) > 0

    # per-class precompute
    pts_f64 = points.astype(np.float64)  # [C, P, 3]
    nrm = (pts_f64**2).sum(-1)  # [C, P]

    in_maps = []
    for k in range(NCORES):
        im = {}
        if G > 0:
            import ml_dtypes
            lhs = np.zeros((G, 128, P), dtype=ml_dtypes.bfloat16)
            rhs = np.zeros((G, 128, P), dtype=ml_dtypes.bfloat16)
            for si, n in enumerate(sym_items[k::NCORES]):
                g, s = si // 4, si % 4
                c = cls[n]
                R = Rp[n].T @ Rg[n]
                z = pts_f64[c] @ R.T  # [P, 3]
                base = 32 * s
                lhs[g, base : base + 3, :] = -2.0 * pts_f64[c].T
                lhs[g, base + 3, :] = 1.0
                lhs[g, base + 4, :] = nrm[c] - MARGIN
                rhs[g, base : base + 3, :] = z.T
                rhs[g, base + 3, :] = nrm[c]
                rhs[g, base + 4, :] = 1.0
            im["lhs"] = lhs
            im["rhs"] = rhs
        if use_diag:
            rhsd = np.zeros((DIAG_K, P), dtype=np.float32)
            lhsd = np.zeros((DIAG_K, DIAG_SLOTS), dtype=np.float32)
            rhsd[6 * DIAG_SLOTS, :] = 1.0
            for d, n in enumerate(diag_items[k::NCORES]):
                c = cls[n]
                Sm = (Rp[n] - Rg[n]).T @ (Rp[n] - Rg[n])
                p3 = pts_f64[c]  # [P, 3]
                r = 6 * d
                rhsd[r + 0, :] = p3[:, 0] ** 2
                rhsd[r + 1, :] = p3[:, 1] ** 2
                rhsd[r + 2, :] = p3[:, 2] ** 2
                rhsd[r + 3, :] = p3[:, 0] * p3[:, 1]
                rhsd[r + 4, :] = p3[:, 0] * p3[:, 2]
                rhsd[r + 5, :] = p3[:, 1] * p3[:, 2]
                lhsd[r + 0, d] = Sm[0, 0]
                lhsd[r + 1, d] = Sm[1, 1]
                lhsd[r + 2, d] = Sm[2, 2]
                lhsd[r + 3, d] = 2.0 * Sm[0, 1]
                lhsd[r + 4, d] = 2.0 * Sm[0, 2]
                lhsd[r + 5, d] = 2.0 * Sm[1, 2]
                lhsd[6 * DIAG_SLOTS, d] = -MARGIN
            im["rhsd"] = rhsd
            im["lhsd"] = lhsd
        in_maps.append(im)
    return in_maps, G, use_diag


def kernel(poses_pred, poses_target, poses_weight, points, symmetry):
    in_maps, G, use_diag = _prepare(
        poses_pred, poses_target, poses_weight, points, symmetry
    )
    if G == 0 and not use_diag:
        return np.float32(0.0)

    run, _nc = _get_runner(G, use_diag)
    results = run(in_maps)

    total = 0.0
    for k in range(NCORES):
        if G > 0:
            total += results[k]["osym"].astype(np.float64).sum()
        if use_diag:
            total += results[k]["odiag"].astype(np.float64).sum()
    return np.float32(0.5 * total / (B * P))


# revision 8
# speedup vs baseline: 1.1318x; 1.0009x over previous
"""Trainium2 Bass kernel for nn_AverageDistanceLoss.

Math: for batch item n with class c, unit quats qp/qg and model points
pts = points[c] ([P,3]):
  d_diag[p] = |Rp pts_p - Rg pts_p|^2 = pts_p^T S pts_p,  S = (Rp-Rg)^T (Rp-Rg)
  d_sym[p]  = min_q |Rp pts_p - Rg pts_q|^2
            = n_p + min_q (n_q - 2 pts_p^T R pts_q),      R = Rp^T Rg
(rotations preserve norms, n_p = |pts_p|^2). The heavy work per symmetric
item is a [P,P] pairwise matmul + row-min; everything index/quaternion
sized (O(B*C), O(B*P)) is host-side prep for sharding.

Sharding: data-parallel over batch. Symmetric items are dealt round-robin
across the 8 cores and packed 4-per-128-partition-group so the K=5
matmuls use 4 concurrent PE row-groups (tile_position). Non-symmetric
items need only d_diag, batched into a single K=6*16 block-diagonal
matmul per core. Each core emits per-point hinge values; the final
scalar reduction happens on host.
"""

import math

import numpy as np

NUM_CLASSES = 22
MARGIN = 0.01
B = 128
P = 1024
NCORES = 8
DIAG_SLOTS = 16  # max non-sym items per core (128/8)
DIAG_K = 6 * DIAG_SLOTS + 1  # 6 quadratic-form rows per item + one margin row

_RUNNER_CACHE: dict = {}


def _quat_to_rotmat(q):
    # q: [..., 4] (w, x, y, z) -> [..., 3, 3], float64
    w, x, y, z = q[..., 0], q[..., 1], q[..., 2], q[..., 3]
    r = np.empty(q.shape[:-1] + (3, 3), dtype=np.float64)
    r[..., 0, 0] = 1 - 2 * (y * y + z * z)
    r[..., 0, 1] = 2 * (x * y - z * w)
    r[..., 0, 2] = 2 * (x * z + y * w)
    r[..., 1, 0] = 2 * (x * y + z * w)
    r[..., 1, 1] = 1 - 2 * (x * x + z * z)
    r[..., 1, 2] = 2 * (y * z - x * w)
    r[..., 2, 0] = 2 * (x * z - y * w)
    r[..., 2, 1] = 2 * (y * z + x * w)
    r[..., 2, 2] = 1 - 2 * (x * x + y * y)
    return r


def _register_pair_min_reduce():
    """Author a custom DVE op: out = min(in0, in1) elementwise, accum_out =
    min-fold of that over the free dim. One 512-cycle VectorE pass consumes
    both PSUM banks of a G row-tile (rd0 = PSUM, rd1 = SBUF staged by ACT)
    and emits the [128,1] row-min directly."""
    import concourse.dve_ops as dve_ops_mod
    from concourse.dve_ops import DveOp
    from concourse.dve_spec import C0, Spec, Src0, Src1, minn, lower as dve_lower
    from concourse.dve_uop import DveOpSpec

    name = "PAIR_MIN_REDUCE_ANT"
    if name in dve_ops_mod._SUB_OPCODE_FOR_NAME:
        return next(op for op in dve_ops_mod.OPS if op.name == name)
    spec = Spec(body=minn(Src0, Src1), accum=minn, accum_init=C0)
    row = dve_ops_mod._CUSTOM_DVE_ROW_BASE + len(dve_ops_mod.OPS)
    assert row < 0x20
    dve_ops_mod._SUB_OPCODE_FOR_NAME[name] = row
    shas = {}
    for ver in ("v3", "v4"):
        uops = dve_lower(spec, ver=ver)
        shas[ver] = DveOpSpec(name=name, opcode=row, uops=uops, rd1_en=True).sha(ver)
    op = DveOp(name, spec, subdim=False, uops_sha=shas)
    dve_ops_mod.OPS.append(op)
    dve_ops_mod.CUSTOM_DVE_SPECS[name] = spec
    return op


def _get_runner(n_groups: int, use_diag: bool):
    key = (n_groups, use_diag)
    if key in _RUNNER_CACHE:
        return _RUNNER_CACHE[key]

    import concourse.bass as bass
    import concourse.tile as tile
    from concourse import bacc, mybir
    from concourse.bass_utils import run_bass_kernel_spmd

    f32 = mybir.dt.float32
    bf16 = mybir.dt.bfloat16
    G = n_groups

    nc = bacc.Bacc("TRN2", target_bir_lowering=False, debug=False)

    ins = {}
    outs = {}
    if G > 0:
        ins["lhs"] = nc.dram_tensor("lhs", [G, 128, P], bf16, kind="ExternalInput").ap()
        ins["rhs"] = nc.dram_tensor("rhs", [G, 128, P], bf16, kind="ExternalInput").ap()
        outs["osym"] = nc.dram_tensor(
            "osym", [G, 128, 32], f32, kind="ExternalOutput"
        ).ap()
    if use_diag:
        ins["rhsd"] = nc.dram_tensor(
            "rhsd", [DIAG_K, P], f32, kind="ExternalInput"
        ).ap()
        ins["lhsd"] = nc.dram_tensor(
            "lhsd", [DIAG_K, DIAG_SLOTS], f32, kind="ExternalInput"
        ).ap()
        outs["odiag"] = nc.dram_tensor(
            "odiag", [DIAG_SLOTS, 1], f32, kind="ExternalOutput"
        ).ap()

    with tile.TileContext(nc) as tc:
        with (
            tc.tile_pool(name="big", bufs=2) as big,
            tc.tile_pool(name="route", bufs=5) as route,
            tc.tile_pool(name="small", bufs=3) as small,
            tc.tile_pool(name="psum", bufs=3, space=bass.MemorySpace.PSUM) as psum,
            tc.tile_pool(name="psumd", bufs=1, space=bass.MemorySpace.PSUM) as psumd,
        ):
            if use_diag:
                rd = small.tile([DIAG_K, P], f32, tag="rd")
                nc.sync.dma_start(rd[:], ins["rhsd"][:])
                ld = small.tile([DIAG_K, DIAG_SLOTS], f32, tag="ld")
                nc.sync.dma_start(ld[:], ins["lhsd"][:])
                pd = psumd.tile([DIAG_SLOTS, P], f32)
                for j in range(2):
                    nc.tensor.matmul(
                        pd[:, 512 * j : 512 * (j + 1)],
                        ld[:],
                        rd[:, 512 * j : 512 * (j + 1)],
                        start=True,
                        stop=True,
                    )
                hd = small.tile([DIAG_SLOTS, P], f32, tag="hd")
                dsum = small.tile([DIAG_SLOTS, 1], f32, tag="dsum")
                nc.scalar.activation(
                    hd[:],
                    pd[:],
                    mybir.ActivationFunctionType.Relu,
                    accum_out=dsum[:],
                )
                nc.sync.dma_start(outs["odiag"][:], dsum[:])

            MINOP = _register_pair_min_reduce()
            biginit = small.tile([128, 1], f32, tag="biginit")
            nc.vector.memset(biginit[:], 3.0e38)
            trash = None
            for g in range(G):
                L = big.tile([128, P], bf16, tag="L")
                nc.sync.dma_start(L[:], ins["lhs"][g][:])
                R = big.tile([128, P], bf16, tag="R")
                nc.sync.dma_start(R[:], ins["rhs"][g][:])
                mins = small.tile([128, 32], f32, tag="mins")
                if trash is None:
                    trash = small.tile([128, 512], f32, tag="trash")
                for m in range(8):
                    for s in range(4):
                        pg = psum.tile([128, P], f32, tag="pg")
                        lT = L[32 * s : 32 * s + 5, 128 * m : 128 * (m + 1)]
                        for j in range(2):
                            nc.tensor.matmul(
                                pg[:, 512 * j : 512 * (j + 1)],
                                lT,
                                R[32 * s : 32 * s + 5, 512 * j : 512 * (j + 1)],
                                start=True,
                                stop=True,
                                tile_position=(32 * s, 0),
                            )
                        col = 8 * s + m
                        c1 = route.tile([128, 512], f32, tag="cp")
                        nc.scalar.copy(c1[:], pg[:, 512:1024])
                        nc.vector._custom_dve(
                            MINOP,
                            out=trash[:],
                            in0=pg[:, 0:512],
                            in1=c1[:],
                            s0=biginit[:],
                            accum_out=mins[:, col : col + 1],
                        )
                hs = small.tile([128, 32], f32, tag="hs")
                nc.scalar.activation(
                    hs[:], mins[:], mybir.ActivationFunctionType.Relu
                )
                nc.sync.dma_start(outs["osym"][g][:], hs[:])

    nc.compile()

    def run(in_maps):
        res = run_bass_kernel_spmd(nc, in_maps, list(range(NCORES)))
        return res.results

    runner = (run, nc)
    _RUNNER_CACHE[key] = runner
    return runner


def _prepare(poses_pred, poses_target, poses_weight, points, symmetry):
    """Host-side shard prep. Returns (in_maps, n_groups, use_diag)."""
    poses_pred = np.asarray(poses_pred, dtype=np.float32)
    poses_target = np.asarray(poses_target, dtype=np.float32)
    poses_weight = np.asarray(poses_weight, dtype=np.float32)
    points = np.asarray(points, dtype=np.float32)
    symmetry = np.asarray(symmetry, dtype=np.float32)

    w = poses_weight.reshape(B, NUM_CLASSES, 4)[:, :, 0]
    has = w > 0
    valid = has.any(axis=1)
    cls = np.argmax(has, axis=1)
    sym = symmetry[cls] > 0

    rows = np.arange(B)
    qp = poses_pred.reshape(B, NUM_CLASSES, 4)[rows, cls].astype(np.float64)
    qg = poses_target.reshape(B, NUM_CLASSES, 4)[rows, cls].astype(np.float64)
    Rp = _quat_to_rotmat(qp)
    Rg = _quat_to_rotmat(qg)

    sym_items = [n for n in range(B) if valid[n] and sym[n]]
    diag_items = [n for n in range(B) if valid[n] and not sym[n]]

    S_max = max((len(sym_items[k::NCORES]) for k in range(NCORES)), default=0)
    G = (S_max + 3) // 4
    use_diag = len(diag_items) > 0

    # per-class precompute
    pts_f64 = points.astype(np.float64)  # [C, P, 3]
    nrm = (pts_f64**2).sum(-1)  # [C, P]

    in_maps = []
    for k in range(NCORES):
        im = {}
        if G > 0:
            import ml_dtypes
            lhs = np.zeros((G, 128, P), dtype=ml_dtypes.bfloat16)
            rhs = np.zeros((G, 128, P), dtype=ml_dtypes.bfloat16)
            for si, n in enumerate(sym_items[k::NCORES]):
                g, s = si // 4, si % 4
                c = cls[n]
                R = Rp[n].T @ Rg[n]
                z = pts_f64[c] @ R.T  # [P, 3]
                base = 32 * s
                lhs[g, base : base + 3, :] = -2.0 * pts_f64[c].T
                lhs[g, base + 3, :] = 1.0
                lhs[g, base + 4, :] = nrm[c] - MARGIN
                rhs[g, base : base + 3, :] = z.T
                rhs[g, base + 3, :] = nrm[c]
                rhs[g, base + 4, :] = 1.0
            im["lhs"] = lhs
            im["rhs"] = rhs
        if use_diag:
            rhsd = np.zeros((DIAG_K, P), dtype=np.float32)
            lhsd = np.zeros((DIAG_K, DIAG_SLOTS), dtype=np.float32)
            rhsd[6 * DIAG_SLOTS, :] = 1.0
            for d, n in enumerate(diag_items[k::NCORES]):
                c = cls[n]
                Sm = (Rp[n] - Rg[n]).T @ (Rp[n] - Rg[n])
                p3 = pts_f64[c]  # [P, 3]
                r = 6 * d
                rhsd[r + 0, :] = p3[:, 0] ** 2
                rhsd[r + 1, :] = p3[:, 1] ** 2
                rhsd[r + 2, :] = p3[:, 2] ** 2
                rhsd[r + 3, :] = p3[:, 0] * p3[:, 1]
                rhsd[r + 4, :] = p3[:, 0] * p3[:, 2]
                rhsd[r + 5, :] = p3[:, 1] * p3[:, 2]
                lhsd[r + 0, d] = Sm[0, 0]
                lhsd[r + 1, d] = Sm[1, 1]
                lhsd[r + 2, d] = Sm[2, 2]
                lhsd[r + 3, d] = 2.0 * Sm[0, 1]
                lhsd[r + 4, d] = 2.0 * Sm[0, 2]
                lhsd[r + 5, d] = 2.0 * Sm[1, 2]
                lhsd[6 * DIAG_SLOTS, d] = -MARGIN
            im["rhsd"] = rhsd
            im["lhsd"] = lhsd
        in_maps.append(im)
    return in_maps, G, use_diag


def kernel(poses_pred, poses_target, poses_weight, points, symmetry):
    in_maps, G, use_diag = _prepare(
        poses_pred, poses_target, poses_weight, points, symmetry
    )
    if G == 0 and not use_diag:
        return np.float32(0.0)

    run, _nc = _get_runner(G, use_diag)
    results = run(in_maps)

    total = 0.0
    for k in range(NCORES):
        if G > 0:
            total += results[k]["osym"].astype(np.float64).sum()
        if use_diag:
            total += results[k]["odiag"].astype(np.float64).sum()
    return np.float32(0.5 * total / (B * P))


# revision 10
# speedup vs baseline: 1.3151x; 1.1620x over previous
"""Trainium2 Bass kernel for nn_AverageDistanceLoss.

Math: for batch item n with class c, unit quats qp/qg and model points
pts = points[c] ([P,3]):
  d_diag[p] = |Rp pts_p - Rg pts_p|^2 = pts_p^T S pts_p,  S = (Rp-Rg)^T (Rp-Rg)
  d_sym[p]  = min_q |Rp pts_p - Rg pts_q|^2
            = n_p + min_q (n_q - 2 pts_p^T R pts_q),      R = Rp^T Rg
(rotations preserve norms, n_p = |pts_p|^2). The heavy work per symmetric
item is a [P,P] pairwise matmul + row-min; everything index/quaternion
sized (O(B*C), O(B*P)) is host-side prep for sharding.

Sharding: data-parallel over batch. Symmetric items are dealt round-robin
across the 8 cores and packed 4-per-128-partition-group so the K=5 bf16
matmuls use 4 concurrent PE row-groups (tile_position). The row-min of
each [128,1024] G tile is computed by a custom DVE op (registered at
runtime): ACT stages one PSUM bank to SBUF, then one VectorE pass reads
PSUM+SBUF in parallel (rd0/rd1), pair-mins them, and min-folds into the
accumulator -> [128,1] per tile. Non-symmetric items need only d_diag,
batched into a single K=97 block-diagonal fp32 matmul per core. Each
core emits per-point hinge values; the final scalar sum happens on host.
"""

import numpy as np

NUM_CLASSES = 22
MARGIN = 0.01
B = 128
P = 1024
NCORES = 8
DIAG_SLOTS = 16  # max non-sym items per core (128/8)
DIAG_K = 6 * DIAG_SLOTS + 1  # 6 quadratic-form rows per item + one margin row

_RUNNER_CACHE: dict = {}


def _quat_to_rotmat(q):
    # q: [..., 4] (w, x, y, z) -> [..., 3, 3], float64
    w, x, y, z = q[..., 0], q[..., 1], q[..., 2], q[..., 3]
    r = np.empty(q.shape[:-1] + (3, 3), dtype=np.float64)
    r[..., 0, 0] = 1 - 2 * (y * y + z * z)
    r[..., 0, 1] = 2 * (x * y - z * w)
    r[..., 0, 2] = 2 * (x * z + y * w)
    r[..., 1, 0] = 2 * (x * y + z * w)
    r[..., 1, 1] = 1 - 2 * (x * x + z * z)
    r[..., 1, 2] = 2 * (y * z - x * w)
    r[..., 2, 0] = 2 * (x * z - y * w)
    r[..., 2, 1] = 2 * (y * z + x * w)
    r[..., 2, 2] = 1 - 2 * (x * x + y * y)
    return r


def _register_pair_min_reduce():
    """Author a custom DVE op: out = min(in0, in1) elementwise, accum_out =
    min-fold of that over the free dim. One 512-cycle VectorE pass consumes
    both PSUM banks of a G row-tile (rd0 = PSUM, rd1 = SBUF staged by ACT)
    and emits the [128,1] row-min directly."""
    import concourse.dve_ops as dve_ops_mod
    from concourse.dve_ops import DveOp
    from concourse.dve_spec import C0, Spec, Src0, Src1, minn, lower as dve_lower
    from concourse.dve_uop import DveOpSpec

    name = "PAIR_MIN_REDUCE_ANT"
    if name in dve_ops_mod._SUB_OPCODE_FOR_NAME:
        return next(op for op in dve_ops_mod.OPS if op.name == name)
    spec = Spec(body=minn(Src0, Src1), accum=minn, accum_init=C0)
    row = dve_ops_mod._CUSTOM_DVE_ROW_BASE + len(dve_ops_mod.OPS)
    assert row < 0x20
    dve_ops_mod._SUB_OPCODE_FOR_NAME[name] = row
    shas = {}
    for ver in ("v3", "v4"):
        uops = dve_lower(spec, ver=ver)
        shas[ver] = DveOpSpec(name=name, opcode=row, uops=uops, rd1_en=True).sha(ver)
    op = DveOp(name, spec, subdim=False, uops_sha=shas)
    dve_ops_mod.OPS.append(op)
    dve_ops_mod.CUSTOM_DVE_SPECS[name] = spec
    return op


def _get_runner(n_slots: int, use_diag: bool):
    key = (n_slots, use_diag)
    if key in _RUNNER_CACHE:
        return _RUNNER_CACHE[key]

    import concourse.bass as bass
    import concourse.tile as tile
    from concourse import bacc, mybir
    from concourse.bass_utils import run_bass_kernel_spmd

    f32 = mybir.dt.float32
    bf16 = mybir.dt.bfloat16
    G = (n_slots + 3) // 4

    nc = bacc.Bacc("TRN2", target_bir_lowering=False, debug=False)

    ins = {}
    outs = {}
    if G > 0:
        ins["lhs"] = nc.dram_tensor("lhs", [G, 128, P], bf16, kind="ExternalInput").ap()
        ins["rhs"] = nc.dram_tensor("rhs", [G, 128, P], bf16, kind="ExternalInput").ap()
        outs["osym"] = nc.dram_tensor(
            "osym", [G, 128, 32], f32, kind="ExternalOutput"
        ).ap()
    if use_diag:
        ins["rhsd"] = nc.dram_tensor(
            "rhsd", [DIAG_K, P], f32, kind="ExternalInput"
        ).ap()
        ins["lhsd"] = nc.dram_tensor(
            "lhsd", [DIAG_K, DIAG_SLOTS], f32, kind="ExternalInput"
        ).ap()
        outs["odiag"] = nc.dram_tensor(
            "odiag", [DIAG_SLOTS, 1], f32, kind="ExternalOutput"
        ).ap()

    with tile.TileContext(nc) as tc:
        with (
            tc.tile_pool(name="big", bufs=2) as big,
            tc.tile_pool(name="route", bufs=5) as route,
            tc.tile_pool(name="small", bufs=3) as small,
            tc.tile_pool(name="psum", bufs=3, space=bass.MemorySpace.PSUM) as psum,
            tc.tile_pool(name="psumd", bufs=1, space=bass.MemorySpace.PSUM) as psumd,
        ):
            if use_diag:
                rd = small.tile([DIAG_K, P], f32, tag="rd")
                nc.sync.dma_start(rd[:], ins["rhsd"][:])
                ld = small.tile([DIAG_K, DIAG_SLOTS], f32, tag="ld")
                nc.sync.dma_start(ld[:], ins["lhsd"][:])
                pd = psumd.tile([DIAG_SLOTS, P], f32)
                for j in range(2):
                    nc.tensor.matmul(
                        pd[:, 512 * j : 512 * (j + 1)],
                        ld[:],
                        rd[:, 512 * j : 512 * (j + 1)],
                        start=True,
                        stop=True,
                    )
                hd = small.tile([DIAG_SLOTS, P], f32, tag="hd")
                dsum = small.tile([DIAG_SLOTS, 1], f32, tag="dsum")
                nc.scalar.activation(
                    hd[:],
                    pd[:],
                    mybir.ActivationFunctionType.Relu,
                    accum_out=dsum[:],
                )
                nc.sync.dma_start(outs["odiag"][:], dsum[:])

            MINOP = _register_pair_min_reduce()
            biginit = small.tile([128, 1], f32, tag="biginit")
            nc.vector.memset(biginit[:], 3.0e38)
            trash = None
            for g in range(G):
                L = big.tile([128, P], bf16, tag="L")
                nc.sync.dma_start(L[:], ins["lhs"][g][:])
                R = big.tile([128, P], bf16, tag="R")
                nc.sync.dma_start(R[:], ins["rhs"][g][:])
                mins = small.tile([128, 32], f32, tag="mins")
                active = [s for s in range(4) if 4 * g + s < n_slots]
                if len(active) < 4:
                    nc.vector.memset(mins[:], 0.0)
                if trash is None:
                    trash = small.tile([128, 512], f32, tag="trash")
                for m in range(8):
                    for s in active:
                        pg = psum.tile([128, P], f32, tag="pg")
                        lT = L[32 * s : 32 * s + 5, 128 * m : 128 * (m + 1)]
                        for j in range(2):
                            nc.tensor.matmul(
                                pg[:, 512 * j : 512 * (j + 1)],
                                lT,
                                R[32 * s : 32 * s + 5, 512 * j : 512 * (j + 1)],
                                start=True,
                                stop=True,
                                tile_position=(32 * s, 0),
                            )
                        col = 8 * s + m
                        c1 = route.tile([128, 512], f32, tag="cp")
                        nc.scalar.copy(c1[:], pg[:, 512:1024])
                        nc.vector._custom_dve(
                            MINOP,
                            out=trash[:],
                            in0=pg[:, 0:512],
                            in1=c1[:],
                            s0=biginit[:],
                            accum_out=mins[:, col : col + 1],
                        )
                hs = small.tile([128, 32], f32, tag="hs")
                nc.scalar.activation(
                    hs[:], mins[:], mybir.ActivationFunctionType.Relu
                )
                nc.sync.dma_start(outs["osym"][g][:], hs[:])

    nc.compile()

    def run(in_maps):
        res = run_bass_kernel_spmd(nc, in_maps, list(range(NCORES)))
        return res.results

    runner = (run, nc)
    _RUNNER_CACHE[key] = runner
    return runner


def _prepare(poses_pred, poses_target, poses_weight, points, symmetry):
    """Host-side shard prep. Returns (in_maps, n_groups, use_diag)."""
    poses_pred = np.asarray(poses_pred, dtype=np.float32)
    poses_target = np.asarray(poses_target, dtype=np.float32)
    poses_weight = np.asarray(poses_weight, dtype=np.float32)
    points = np.asarray(points, dtype=np.float32)
    symmetry = np.asarray(symmetry, dtype=np.float32)

    w = poses_weight.reshape(B, NUM_CLASSES, 4)[:, :, 0]
    has = w > 0
    valid = has.any(axis=1)
    cls = np.argmax(has, axis=1)
    sym = symmetry[cls] > 0

    rows = np.arange(B)
    qp = poses_pred.reshape(B, NUM_CLASSES, 4)[rows, cls].astype(np.float64)
    qg = poses_target.reshape(B, NUM_CLASSES, 4)[rows, cls].astype(np.float64)
    Rp = _quat_to_rotmat(qp)
    Rg = _quat_to_rotmat(qg)

    sym_items = [n for n in range(B) if valid[n] and sym[n]]
    diag_items = [n for n in range(B) if valid[n] and not sym[n]]

    S_max = max((len(sym_items[k::NCORES]) for k in range(NCORES)), default=0)
    G = (S_max + 3) // 4
    use_diag = len(diag_items) > 0

    # per-class precompute
    pts_f64 = points.astype(np.float64)  # [C, P, 3]
    nrm = (pts_f64**2).sum(-1)  # [C, P]

    in_maps = []
    for k in range(NCORES):
        im = {}
        if G > 0:
            import ml_dtypes
            lhs = np.zeros((G, 128, P), dtype=ml_dtypes.bfloat16)
            rhs = np.zeros((G, 128, P), dtype=ml_dtypes.bfloat16)
            for si, n in enumerate(sym_items[k::NCORES]):
                g, s = si // 4, si % 4
                c = cls[n]
                R = Rp[n].T @ Rg[n]
                z = pts_f64[c] @ R.T  # [P, 3]
                base = 32 * s
                lhs[g, base : base + 3, :] = -2.0 * pts_f64[c].T
                lhs[g, base + 3, :] = 1.0
                lhs[g, base + 4, :] = nrm[c] - MARGIN
                rhs[g, base : base + 3, :] = z.T
                rhs[g, base + 3, :] = nrm[c]
                rhs[g, base + 4, :] = 1.0
            im["lhs"] = lhs
            im["rhs"] = rhs
        if use_diag:
            rhsd = np.zeros((DIAG_K, P), dtype=np.float32)
            lhsd = np.zeros((DIAG_K, DIAG_SLOTS), dtype=np.float32)
            rhsd[6 * DIAG_SLOTS, :] = 1.0
            for d, n in enumerate(diag_items[k::NCORES]):
                c = cls[n]
                Sm = (Rp[n] - Rg[n]).T @ (Rp[n] - Rg[n])
                p3 = pts_f64[c]  # [P, 3]
                r = 6 * d
                rhsd[r + 0, :] = p3[:, 0] ** 2
                rhsd[r + 1, :] = p3[:, 1] ** 2
                rhsd[r + 2, :] = p3[:, 2] ** 2
                rhsd[r + 3, :] = p3[:, 0] * p3[:, 1]
                rhsd[r + 4, :] = p3[:, 0] * p3[:, 2]
                rhsd[r + 5, :] = p3[:, 1] * p3[:, 2]
                lhsd[r + 0, d] = Sm[0, 0]
                lhsd[r + 1, d] = Sm[1, 1]
                lhsd[r + 2, d] = Sm[2, 2]
                lhsd[r + 3, d] = 2.0 * Sm[0, 1]
                lhsd[r + 4, d] = 2.0 * Sm[0, 2]
                lhsd[r + 5, d] = 2.0 * Sm[1, 2]
                lhsd[6 * DIAG_SLOTS, d] = -MARGIN
            im["rhsd"] = rhsd
            im["lhsd"] = lhsd
        in_maps.append(im)
    return in_maps, S_max, use_diag


def kernel(poses_pred, poses_target, poses_weight, points, symmetry):
    in_maps, S_max, use_diag = _prepare(
        poses_pred, poses_target, poses_weight, points, symmetry
    )
    if S_max == 0 and not use_diag:
        return np.float32(0.0)

    run, _nc = _get_runner(S_max, use_diag)
    results = run(in_maps)

    G = (S_max + 3) // 4
    total = 0.0
    for k in range(NCORES):
        if G > 0:
            total += results[k]["osym"].astype(np.float64).sum()
        if use_diag:
            total += results[k]["odiag"].astype(np.float64).sum()
    return np.float32(0.5 * total / (B * P))
